# revision 1
# baseline (speedup 1.0000x reference)
"""BalancedMSELoss (contrastive-style) Trainium2 kernel.

loss = 2 * mean_i( logsumexp_j(-0.5*(p_i - t_j)^2) + 0.5*(p_i - t_i)^2 )

The O(N^2) part — S_i = sum_j exp(-0.5*(p_i - t_j)^2) — runs on 8
NeuronCores, rows of the NxN grid sharded across cores (2048 rows each).

Per core:
  - TensorE: -0.5*(p-t)^2 = -0.5*p^2 + p*t - 0.5*t^2 is a rank-3 matmul:
      lhsT = [p^2; p; 1] (K=3 x M=128 row block), rhs = [-0.5; t; -0.5*t^2]
      (K=3 x N=512 col block) -> PSUM [128, 512].
  - ScalarE: Exp over PSUM chunks of [128, 2048] with accum_out producing
      per-row partial sums directly (no DVE reduce over the big grid).
  - VectorE: tiny [128, 8] partial-sum reductions only.

Host does the O(N) tail in float64: lse = log(S), diagonal term, mean.
"""

import numpy as np

N = 16384
NCORES = 8
ROWS_PER_CORE = N // NCORES          # 2048
RB = ROWS_PER_CORE // 128            # 16 row blocks per core
JCH = 2048                           # columns per ACT chunk (4 PSUM banks)
NJ = N // JCH                        # 8 chunks per row block
MM_N = 512                           # matmul free dim (1 PSUM bank, fp32 max)

_CACHE = {}


def _build_nc():
    import concourse.bacc as bacc
    import concourse.bass as bass
    import concourse.mybir as mybir
    import concourse.tile as tile

    f32 = mybir.dt.float32
    nc = bacc.Bacc(
        "TRN2",
        target_bir_lowering=False,
        debug=False,
        enable_asserts=False,
        num_devices=NCORES,
    )

    lhsT_d = nc.dram_tensor("lhst_in", [3, ROWS_PER_CORE], f32, kind="ExternalInput")
    rhs_d = nc.dram_tensor("rhs_in", [3, N], f32, kind="ExternalInput")
    sout_d = nc.dram_tensor("s_out", [128, RB], f32, kind="ExternalOutput")

    with tile.TileContext(nc) as tc:
        with (
            tc.tile_pool(name="const", bufs=1) as cpool,
            tc.tile_pool(name="psum", bufs=2, space=bass.MemorySpace.PSUM) as ppool,
            tc.tile_pool(name="esb", bufs=2) as epool,
            tc.tile_pool(name="sp", bufs=2) as sppool,
            tc.tile_pool(name="sout", bufs=1) as opool,
        ):
            lhsT = cpool.tile([3, ROWS_PER_CORE], f32)
            rhs = cpool.tile([3, N], f32)
            nc.sync.dma_start(lhsT[:], lhsT_d[:])
            nc.sync.dma_start(rhs[:], rhs_d[:])

            s_out = opool.tile([128, RB], f32)
            for rb in range(RB):
                sp = sppool.tile([128, NJ], f32)
                for jb in range(NJ):
                    d2 = ppool.tile([128, JCH], f32)
                    for q in range(JCH // MM_N):
                        nc.tensor.matmul(
                            d2[:, q * MM_N : (q + 1) * MM_N],
                            lhsT[:, rb * 128 : (rb + 1) * 128],
                            rhs[:, jb * JCH + q * MM_N : jb * JCH + (q + 1) * MM_N],
                            start=True,
                            stop=True,
                        )
                    e = epool.tile([128, JCH], f32)
                    nc.scalar.activation(
                        e[:],
                        d2[:],
                        mybir.ActivationFunctionType.Exp,
                        accum_out=sp[:, jb : jb + 1],
                    )
                nc.vector.reduce_sum(
                    s_out[:, rb : rb + 1], sp[:], axis=mybir.AxisListType.X
                )
            nc.sync.dma_start(sout_d[:], s_out[:])

    nc.compile()
    return nc


def _get_nc():
    if "nc" not in _CACHE:
        _CACHE["nc"] = _build_nc()
    return _CACHE["nc"]


def _make_in_maps(p, t):
    rhs_host = np.ascontiguousarray(
        np.stack([np.full(N, -0.5, np.float32), t, (-0.5 * t * t).astype(np.float32)])
    )
    in_maps = []
    for c in range(NCORES):
        pc = p[c * ROWS_PER_CORE : (c + 1) * ROWS_PER_CORE]
        lhsT_host = np.ascontiguousarray(
            np.stack(
                [
                    (pc * pc).astype(np.float32),
                    pc,
                    np.ones(ROWS_PER_CORE, np.float32),
                ]
            )
        )
        in_maps.append({"lhst_in": lhsT_host, "rhs_in": rhs_host})
    return in_maps


def kernel(inputs, targets, _trace=False):
    from concourse.bass_utils import run_bass_kernel_spmd

    p = np.asarray(inputs, dtype=np.float32).reshape(-1)
    t = np.asarray(targets, dtype=np.float32).reshape(-1)
    assert p.shape == (N,) and t.shape == (N,)

    nc = _get_nc()
    in_maps = _make_in_maps(p, t)
    out = run_bass_kernel_spmd(
        nc, in_maps, core_ids=list(range(NCORES)), trace=_trace
    )
    # s_out[part, rb] holds S for local row rb*128 + part
    S = np.concatenate([out.results[c]["s_out"].T.reshape(-1) for c in range(NCORES)])

    lse = np.log(S.astype(np.float64))
    pd = p.astype(np.float64)
    td = t.astype(np.float64)
    diag = -0.5 * (pd - td) ** 2
    loss = 2.0 * float(np.mean(lse - diag))
    result = np.array(loss, dtype=np.float32)
    if _trace:
        _CACHE["last_exec_time_ns"] = out.exec_time_ns
        _CACHE["last_profile"] = out
    return result


# revision 4
# speedup vs baseline: 43.9343x; 43.9343x over previous
"""BalancedMSELoss (contrastive-style) Trainium2 kernel.

loss = 2 * mean_i( logsumexp_j(-0.5*(p_i - t_j)^2) + 0.5*(p_i - t_i)^2 )

The O(N^2) part — S_i = sum_j exp(-0.5*(p_i - t_j)^2) — runs on 8
NeuronCores, rows of the NxN grid sharded across cores (2048 rows each).

Per core:
  - TensorE: -0.5*(p-t)^2 = -0.5*p^2 + p*t - 0.5*t^2 as a rank-8 matmul.
      fp32 matmuls lower to 2 slow HW matmuls, so all operands are bf16
      hi/lo splits (x = xh + xl exactly in bf16 pairs); p*t = (ph+pl)(th+tl)
      expands to 4 exact bf16 products, -0.5p^2 and -0.5t^2 ride along as
      split constant rows. K=8, N=512 per matmul, fp32 PSUM accumulate:
      precision ~4e-5 worst-case in the exponent, ~2e-9 on the final loss.
  - ScalarE: Exp over PSUM chunks of [128, 2048] with accum_out producing
      per-row partial sums directly (no DVE reduce over the big grid).
  - VectorE: tiny [128, 8] partial-sum reductions only.

Host does the O(N) tail in float64: lse = log(S), diagonal term, mean.
"""

import numpy as np

N = 16384
NCORES = 8
ROWS_PER_CORE = N // NCORES          # 2048
RB = ROWS_PER_CORE // 128            # 16 row blocks per core
JCH = 2048                           # columns per ACT chunk (4 PSUM banks)
NJ = N // JCH                        # 8 chunks per row block
MM_N = 512                           # matmul free dim (1 PSUM bank, fp32 max)

_CACHE = {}


def _build_nc():
    import concourse.bacc as bacc
    import concourse.bass as bass
    import concourse.mybir as mybir
    import concourse.tile as tile

    f32 = mybir.dt.float32
    bf16 = mybir.dt.bfloat16
    nc = bacc.Bacc(
        "TRN2",
        target_bir_lowering=False,
        debug=False,
        enable_asserts=False,
        num_devices=NCORES,
    )

    lhsT_d = nc.dram_tensor("lhst_in", [8, ROWS_PER_CORE], bf16, kind="ExternalInput")
    rhs_d = nc.dram_tensor("rhs_in", [8, N], bf16, kind="ExternalInput")
    sout_d = nc.dram_tensor("s_out", [128, RB], f32, kind="ExternalOutput")

    with tile.TileContext(nc) as tc:
        with (
            tc.tile_pool(name="const", bufs=1) as cpool,
            tc.tile_pool(name="psum", bufs=2, space=bass.MemorySpace.PSUM) as ppool,
            tc.tile_pool(name="esb", bufs=2) as epool,
            tc.tile_pool(name="sp", bufs=2) as sppool,
            tc.tile_pool(name="sout", bufs=1) as opool,
        ):
            lhsT = cpool.tile([8, ROWS_PER_CORE], bf16)
            rhs = cpool.tile([8, N], bf16)
            nc.sync.dma_start(lhsT[:], lhsT_d[:])
            nc.sync.dma_start(rhs[:], rhs_d[:])

            s_out = opool.tile([128, RB], f32)
            for rb in range(RB):
                sp = sppool.tile([128, NJ], f32)
                for jb in range(NJ):
                    d2 = ppool.tile([128, JCH], f32)
                    for q in range(JCH // MM_N):
                        nc.tensor.matmul(
                            d2[:, q * MM_N : (q + 1) * MM_N],
                            lhsT[:, rb * 128 : (rb + 1) * 128],
                            rhs[:, jb * JCH + q * MM_N : jb * JCH + (q + 1) * MM_N],
                            start=True,
                            stop=True,
                        )
                    e = epool.tile([128, JCH], f32)
                    nc.scalar.activation(
                        e[:],
                        d2[:],
                        mybir.ActivationFunctionType.Exp,
                        accum_out=sp[:, jb : jb + 1],
                    )
                nc.vector.reduce_sum(
                    s_out[:, rb : rb + 1], sp[:], axis=mybir.AxisListType.X
                )
            nc.sync.dma_start(sout_d[:], s_out[:])

    nc.compile()
    return nc


def _get_nc():
    if "nc" not in _CACHE:
        _CACHE["nc"] = _build_nc()
    return _CACHE["nc"]


def _split_bf16(x):
    import ml_dtypes

    hi = x.astype(ml_dtypes.bfloat16)
    lo = (x - hi.astype(np.float32)).astype(ml_dtypes.bfloat16)
    return hi, lo


def _make_in_maps(p, t):
    import ml_dtypes

    bf = ml_dtypes.bfloat16
    f = np.float32
    th, tl = _split_bf16(t)
    qth, qtl = _split_bf16((f(-0.5) * t * t).astype(f))
    ones_t = np.ones(N, bf)
    # row k of lhsT pairs with row k of rhs; their products sum to
    # p*t - 0.5p^2 - 0.5t^2 = -0.5(p-t)^2
    rhs_host = np.ascontiguousarray(
        np.stack([th, th, tl, tl, ones_t, ones_t, qth, qtl])
    )
    in_maps = []
    for c in range(NCORES):
        pc = p[c * ROWS_PER_CORE : (c + 1) * ROWS_PER_CORE]
        ph, pl = _split_bf16(pc)
        qph, qpl = _split_bf16((f(-0.5) * pc * pc).astype(f))
        ones_p = np.ones(ROWS_PER_CORE, bf)
        lhsT_host = np.ascontiguousarray(
            np.stack([ph, pl, ph, pl, qph, qpl, ones_p, ones_p])
        )
        in_maps.append({"lhst_in": lhsT_host, "rhs_in": rhs_host})
    return in_maps


def kernel(inputs, targets, _trace=False):
    from concourse.bass_utils import run_bass_kernel_spmd

    p = np.asarray(inputs, dtype=np.float32).reshape(-1)
    t = np.asarray(targets, dtype=np.float32).reshape(-1)
    assert p.shape == (N,) and t.shape == (N,)

    nc = _get_nc()
    in_maps = _make_in_maps(p, t)
    out = run_bass_kernel_spmd(
        nc, in_maps, core_ids=list(range(NCORES)), trace=_trace
    )
    # s_out[part, rb] holds S for local row rb*128 + part
    S = np.concatenate([out.results[c]["s_out"].T.reshape(-1) for c in range(NCORES)])

    lse = np.log(S.astype(np.float64))
    pd = p.astype(np.float64)
    td = t.astype(np.float64)
    diag = -0.5 * (pd - td) ** 2
    loss = 2.0 * float(np.mean(lse - diag))
    result = np.array(loss, dtype=np.float32)
    if _trace:
        _CACHE["last_exec_time_ns"] = out.exec_time_ns
        _CACHE["last_profile"] = out
    return result
